# revision 1
# baseline (speedup 1.0000x reference)
"""Trainium2 Bass kernel for MoE-with-LoRA-experts (nn_MoE_64098091925598).

Reference computation (N=8192 tokens, D=1024, E=8 experts, R=16, top-2):
    logits  = x @ W_gate.T                      [N, E]
    combine = scatter(softmax(top2(logits)))    [N, E] (2 nonzeros/row)
    moe     = sum_e combine[:,e] * (x @ A_e @ B_e)
    out     = moe + x @ W_base.T + b_base

Strategy: data-parallel over tokens across 8 NeuronCores (1024 tokens
per core); every core computes all 8 LoRA experts densely (the expert
FLOPs are tiny vs the base linear, so expert-parallel all-to-all would
be pure overhead) and masks by the combine weights. All matmuls run in
float32r (TF32-like fast fp32 path, full PE rate at N>=256).

Key algebraic trick: with H[n,(e,r)] = combine[n,e] * (x @ A_e)[n,r]
stacked over experts, the weighted expert sum collapses to a single
dense K=128 matmul  moe = H @ B_flat,  which accumulates into the same
PSUM tile as the base-linear matmuls.

Layouts (host-prepped): x is transposed to xT [D, N] so the contraction
dim d sits on SBUF partitions; outputs are produced token-major [n, d]
directly, so the gather is a plain concat.

Perf structure:
- Loads stream in priority order with x^T's first 512-token half split
  across BOTH HWDGE rings (sync+scalar) so gating starts ~12us in; the
  W_base halves follow on whichever ring frees up, ordered dt-major to
  feed the dt-outer main loop.
- The gating/LoRA front-end is pipelined per 512-token wave (separate
  tiles per wave so Tile's dependency tracking doesn't serialize).
- A short dummy-matmul burst warms the PE clock gate (HAM) while the
  first loads are in flight.
"""

import numpy as np

import concourse.mybir as mybir
import concourse.tile as tile
from concourse import bacc
from concourse.bass_utils import run_bass_kernel_spmd
from concourse.masks import make_identity

N_TOK, D, E, R, TOPK = 8192, 1024, 8, 16, 2
CORES = 8
NS = N_TOK // CORES  # tokens per core
ER = E * R  # 128, stacked expert-rank dim
DC = D // 128  # 8 contraction chunks
NJ = NS // 128  # 8 token chunks per core
NT = NS // 512  # 2 wide token tiles (waves) per core
JT = NJ // NT  # 4 token chunks per wave
DT = D // 512  # 2 dout tiles

f32 = mybir.dt.float32
f32r = mybir.dt.float32r

N_WARM = 36  # dummy matmuls to warm the PE clock gate during loads

_CACHE: dict = {}


def _kernel_body(nc, tc, dram):
    xT, wbT, a_fl, b_fl, wgT, exp_m, b_vec, out = dram

    from contextlib import ExitStack

    ctx = ExitStack()
    pw = ctx.enter_context(tc.tile_pool(name="weights", bufs=1))
    pg = ctx.enter_context(tc.tile_pool(name="gating", bufs=1))
    pmt = ctx.enter_context(tc.tile_pool(name="mmtmp", bufs=2))
    pout = ctx.enter_context(tc.tile_pool(name="outsb", bufs=4))
    ps_tp = ctx.enter_context(tc.tile_pool(name="ps_tp", bufs=2, space="PSUM"))
    ps_mm = ctx.enter_context(tc.tile_pool(name="ps_mm", bufs=2, space="PSUM"))
    ps_out = ctx.enter_context(tc.tile_pool(name="ps_out", bufs=4, space="PSUM"))

    # ---- PE prewarm: garbage matmuls, no data deps, never read -----
    warm_sb = pw.tile([128, 128], f32r, tag="warm")
    warm_ps = ps_tp.tile([128, 64], f32, tag="tp")
    nc.vector.memset(warm_sb.bitcast(f32), 0.0)
    for _ in range(N_WARM):
        nc.tensor.matmul(
            warm_ps, warm_sb, warm_sb[:, 0:64], start=True, stop=True
        )

    # ---- Load phase: two HWDGE rings, strict priority order ---------
    # ring A = nc.sync, ring B = nc.scalar. x^T wave-0 is split across
    # both rings; then wave-1 + LoRA weights on A while W_base streams
    # on B (dt-major for the dt-outer main loop).
    xT3 = xT.rearrange("(c p) n -> p c n", p=128)
    wbT3 = wbT.rearrange("(c p) d -> p c d", p=128)

    def ring(c):
        return nc.sync if c % 2 == 0 else nc.scalar

    wg_sb = pw.tile([128, DC, E], f32r, tag="wg")
    nc.scalar.dma_start(out=wg_sb, in_=wgT.rearrange("(c p) e -> p c e", p=128))
    exp_sb = pw.tile([E, ER], f32r, tag="expand")
    nc.scalar.dma_start(out=exp_sb, in_=exp_m)

    xt = [[None] * NT for _ in range(DC)]
    for c in range(DC):
        t = pw.tile([128, 512], f32r, tag=f"xt{c}_0")
        ring(c).dma_start(out=t, in_=xT3[:, c, 0:512])
        xt[c][0] = t

    a_sb = pw.tile([128, DC, ER], f32r, tag="a")
    nc.sync.dma_start(out=a_sb, in_=a_fl.rearrange("(c p) r -> p c r", p=128))

    bias_sb = pw.tile([128, D], f32, tag="bias")
    nc.gpsimd.dma_start(out=bias_sb, in_=b_vec.to_broadcast([128, D]))

    b_sb = pw.tile([ER, D], f32r, tag="bflat")
    nc.scalar.dma_start(out=b_sb, in_=b_fl)

    for c in range(DC):
        t = pw.tile([128, 512], f32r, tag=f"xt{c}_1")
        ring(c).dma_start(out=t, in_=xT3[:, c, 512:1024])
        xt[c][1] = t

    wb = [[None] * DT for _ in range(DC)]
    for dt in range(DT):
        for c in range(DC):
            t = pw.tile([128, 512], f32r, tag=f"wb{c}_{dt}")
            ring(c).dma_start(out=t, in_=wbT3[:, c, dt * 512 : (dt + 1) * 512])
            wb[c][dt] = t

    ident = pw.tile([128, 128], f32, tag="ident")
    make_identity(nc, ident)

    # ---- Front-end, pipelined per 512-token wave --------------------
    HT_sb = pg.tile([ER, NS], f32r, tag="HT")
    for t in range(NT):
        sl = slice(t * 512, (t + 1) * 512)

        # logits^T [E, 512] for this wave
        lgT_ps = ps_mm.tile([E, 512], f32, tag="mm")
        for c in range(DC):
            nc.tensor.matmul(
                lgT_ps, wg_sb[:, c, :], xt[c][t], start=(c == 0), stop=(c == DC - 1)
            )
        lgT_sb = pg.tile([E, 512], f32, tag=f"lgT{t}")
        nc.vector.tensor_copy(lgT_sb, lgT_ps)

        # token-major logits chunks + sorted top-8 per token
        lg3 = pg.tile([128, JT, E], f32, tag=f"lg3_{t}")
        mx = pg.tile([128, JT, E], f32, tag=f"mx{t}")
        for r in range(JT):
            tr_ps = ps_tp.tile([128, E], f32, tag="tp")
            nc.tensor.transpose(
                tr_ps, lgT_sb[:, r * 128 : (r + 1) * 128], ident[0:E, 0:E]
            )
            nc.vector.tensor_copy(lg3[:, r, :], tr_ps)
            nc.vector.max(out=mx[:, r, :], in_=lg3[:, r, :])

        # combine = 1{l==v1}*sigmoid(v1-v2) + 1{l==v2}*sigmoid(v2-v1)
        v1 = mx[:, :, 0:1]
        v2 = mx[:, :, 1:2]
        d21 = pg.tile([128, JT, 1], f32, tag=f"d21_{t}")
        nc.vector.tensor_sub(d21, v2, v1)
        w1 = pg.tile([128, JT, 1], f32, tag=f"w1_{t}")
        w2 = pg.tile([128, JT, 1], f32, tag=f"w2_{t}")
        nc.scalar.activation(w2, d21, mybir.ActivationFunctionType.Sigmoid)
        nc.scalar.activation(w1, d21, mybir.ActivationFunctionType.Sigmoid, scale=-1.0)

        eq1 = pg.tile([128, JT, E], f32, tag=f"eq1_{t}")
        eq2 = pg.tile([128, JT, E], f32, tag=f"eq2_{t}")
        cb = pg.tile([128, JT, E], f32, tag=f"cb{t}")
        bs = [128, JT, E]
        nc.vector.tensor_tensor(eq1, lg3, v1.to_broadcast(bs), mybir.AluOpType.is_equal)
        nc.vector.tensor_tensor(eq2, lg3, v2.to_broadcast(bs), mybir.AluOpType.is_equal)
        nc.vector.tensor_tensor(eq1, eq1, w1.to_broadcast(bs), mybir.AluOpType.mult)
        nc.vector.tensor_tensor(eq2, eq2, w2.to_broadcast(bs), mybir.AluOpType.mult)
        nc.vector.tensor_add(cb, eq1, eq2)

        # combine^T [E, 512] via PE transpose per chunk
        cT_sb = pg.tile([E, 512], f32r, tag=f"cT{t}")
        for r in range(JT):
            cT_ps = ps_tp.tile([E, 128], f32, tag="tp")
            nc.tensor.transpose(cT_ps, cb[:, r, :], ident)
            nc.vector.tensor_copy(cT_sb[:, r * 128 : (r + 1) * 128], cT_ps)

        # H^T = (A_flat^T @ x^T) * expand(combine^T)
        h_ps = ps_mm.tile([ER, 512], f32, tag="mm")
        for c in range(DC):
            nc.tensor.matmul(
                h_ps, a_sb[:, c, :], xt[c][t], start=(c == 0), stop=(c == DC - 1)
            )
        h_sb = pmt.tile([ER, 512], f32, tag="hsb")
        nc.vector.tensor_copy(h_sb, h_ps)
        ce_ps = ps_mm.tile([ER, 512], f32, tag="mm")
        nc.tensor.matmul(ce_ps, exp_sb, cT_sb, start=True, stop=True)
        nc.vector.tensor_tensor(HT_sb[:, sl], ce_ps, h_sb, mybir.AluOpType.mult)

    # ---- Main accumulation: out[n,d] = x@Wb^T + H@B_flat + b --------
    for dt in range(DT):
        dsl = slice(dt * 512, (dt + 1) * 512)
        for j in range(NJ):
            jsl = slice(j * 128, (j + 1) * 128)
            jh, jr = divmod(j, JT)
            out_ps = ps_out.tile([128, 512], f32, tag="out")
            for c in range(DC):
                nc.tensor.matmul(
                    out_ps,
                    xt[c][jh][:, jr * 128 : (jr + 1) * 128],
                    wb[c][dt],
                    start=(c == 0),
                    stop=False,
                )
            nc.tensor.matmul(out_ps, HT_sb[:, jsl], b_sb[:, dsl], start=False, stop=True)
            out_sb = pout.tile([128, 512], f32, tag="osb")
            nc.vector.tensor_add(out_sb, out_ps, bias_sb[:, dsl])
            eng = nc.sync if (j + dt) % 2 == 0 else nc.scalar
            eng.dma_start(out=out[jsl, dsl], in_=out_sb)

    ctx.close()


def build_nc():
    nc = bacc.Bacc(
        "TRN2",
        target_bir_lowering=False,
        debug=False,
        enable_asserts=False,
        num_devices=CORES,
    )
    xT = nc.dram_tensor("xT", [D, NS], f32, kind="ExternalInput").ap()
    wbT = nc.dram_tensor("wbT", [D, D], f32, kind="ExternalInput").ap()
    a_fl = nc.dram_tensor("a_fl", [D, ER], f32, kind="ExternalInput").ap()
    b_fl = nc.dram_tensor("b_fl", [ER, D], f32, kind="ExternalInput").ap()
    wgT = nc.dram_tensor("wgT", [D, E], f32, kind="ExternalInput").ap()
    exp_m = nc.dram_tensor("exp_m", [E, ER], f32, kind="ExternalInput").ap()
    b_vec = nc.dram_tensor("b_vec", [1, D], f32, kind="ExternalInput").ap()
    out = nc.dram_tensor("out", [NS, D], f32, kind="ExternalOutput").ap()

    dram = (
        xT.bitcast(f32r),
        wbT.bitcast(f32r),
        a_fl.bitcast(f32r),
        b_fl.bitcast(f32r),
        wgT.bitcast(f32r),
        exp_m.bitcast(f32r),
        b_vec,
        out,
    )
    with tile.TileContext(nc) as tc:
        _kernel_body(nc, tc, dram)
    nc.compile()
    return nc


def host_prep(x, W_gate, A, B, W_base, b_base):
    """Shard + lay out the full inputs into 8 per-core input maps."""
    xT = np.ascontiguousarray(x.T)  # [D, N]
    wbT = np.ascontiguousarray(W_base.T)  # [din, dout]
    a_fl = np.ascontiguousarray(A.transpose(1, 0, 2).reshape(D, ER))
    b_fl = np.ascontiguousarray(B.reshape(ER, D))
    wgT = np.ascontiguousarray(W_gate.T)  # [D, E]
    exp_m = np.zeros((E, ER), dtype=np.float32)
    for e in range(E):
        exp_m[e, e * R : (e + 1) * R] = 1.0
    b_vec = np.ascontiguousarray(b_base.reshape(1, D))

    in_maps = []
    for c in range(CORES):
        in_maps.append(
            {
                "xT": np.ascontiguousarray(xT[:, c * NS : (c + 1) * NS]),
                "wbT": wbT,
                "a_fl": a_fl,
                "b_fl": b_fl,
                "wgT": wgT,
                "exp_m": exp_m,
                "b_vec": b_vec,
            }
        )
    return in_maps


def kernel(x, W_gate, A, B, W_base, b_base):
    x = np.asarray(x, dtype=np.float32)
    W_gate = np.asarray(W_gate, dtype=np.float32)
    A = np.asarray(A, dtype=np.float32)
    B = np.asarray(B, dtype=np.float32)
    W_base = np.asarray(W_base, dtype=np.float32)
    b_base = np.asarray(b_base, dtype=np.float32)

    if "nc" not in _CACHE:
        _CACHE["nc"] = build_nc()
    nc = _CACHE["nc"]

    in_maps = host_prep(x, W_gate, A, B, W_base, b_base)
    res = run_bass_kernel_spmd(nc, in_maps, core_ids=list(range(CORES)))
    return np.concatenate([res.results[c]["out"] for c in range(CORES)], axis=0)

